# revision 11
# baseline (speedup 1.0000x reference)
"""GemmaAttention (B=2, S=2048, D=2048, H=8, KV=1, HD=256) on 8 trn2 NeuronCores.

Sharding: DP=2 over batch x TP=4 over head-pairs. Core c handles batch c//4 and
heads {2*(c%4), 2*(c%4)+1}. Each core computes its partial o_proj output
(row-parallel Wo); the host sums the 4 partials per batch (the all-reduce is
folded into the host-side unshard).

Dataflow on each core (everything float32r on the PE at full rate):
  QT[dq,s]  = Wq_sl.T @ hT   (hT = hidden[b].T, host-transposed)
  KT[dk,s]  = Wk.T   @ hT
  V[s,dv]   = (hT chunks as lhsT) @ Wv
  RoPE applied to QT/KT in the psum->SBUF drain (DVE), with 1/sqrt(HD) folded
  into the exp's scale argument.
  scoresT[k,q] = KT_chunk.T @ QT  (per head)
  expT = ACT Exp(scoresT * 1/16) (+ causal staircase / external mask)
  outT[dv,q] += V_chunk.T @ expT ; denominators via DVE accumulation of expT
  plus a ones-vector matmul partition-reduce; normalize outT by 1/sum.
  out_partial[s,:] = outTn_chunk.T @ Wo_sl   -> DMA to DRAM.
"""

import numpy as np

import concourse.bass as bass
import concourse.tile as tile
import concourse.mybir as mybir
from concourse import bacc
from concourse.bass_utils import run_bass_kernel_spmd
from concourse._compat import with_exitstack  # noqa: F401

P = 128
B, S, D = 2, 2048, 2048
H, KV, HD = 8, 1, 256
ROPE_BASE = 10000.0
NEG_BIG = -1.0e30

HEADS_PER_CORE = 2
DQ = HEADS_PER_CORE * HD          # 512 q-dims per core
DCH = D // P                      # 16 contraction chunks
SBLK = 512                        # s-tile for projection rhs / q-tile
NSBLK = S // SBLK                 # 4
NKC = S // P                      # 16 key chunks
NQCH = DQ // P                    # 4 QT partition chunks
NKCH = HD // P                    # 2 KT partition chunks

F32 = mybir.dt.float32
F32R = mybir.dt.float32r
EXP = mybir.ActivationFunctionType.Exp

# exec time of the last traced run (set by run_spmd when tracing)
LAST_EXEC_TIME_NS = None

_BUILD_CACHE = {}


def _build(causal: bool):
    nc = bacc.Bacc()

    hT = nc.declare_dram_parameter("hT", [D, S], F32R, isOutput=False)
    wq = nc.declare_dram_parameter("wq", [D, DQ], F32R, isOutput=False)
    wk = nc.declare_dram_parameter("wk", [D, HD], F32R, isOutput=False)
    wv = nc.declare_dram_parameter("wv", [D, HD], F32R, isOutput=False)
    wo = nc.declare_dram_parameter("wo", [DQ, D], F32R, isOutput=False)
    cosT = nc.declare_dram_parameter("cosT", [HD, S], F32, isOutput=False)
    sinT = nc.declare_dram_parameter("sinT", [HD, S], F32, isOutput=False)
    ones = nc.declare_dram_parameter("ones", [P, P], F32R, isOutput=False)
    if causal:
        stair = nc.declare_dram_parameter("stair", [P, 2 * SBLK], F32, isOutput=False)
    else:
        maskT = nc.declare_dram_parameter("maskT16", [S, S], F32, isOutput=False)
    outp = nc.declare_dram_parameter("out_partial", [S, D], F32, isOutput=True)

    from contextlib import ExitStack
    with tile.TileContext(nc) as tc, ExitStack() as ctx:
        # persistent across phases: QT/KT/V + small consts
        pq = ctx.enter_context(tc.tile_pool(name="pq", bufs=1))
        QT = pq.tile([P, NQCH, S], F32R, name="QT")
        KT = pq.tile([P, NKCH, S], F32R, name="KT")
        VN = pq.tile([P, NKC, HD], F32R, name="VN")
        ONES = pq.tile([P, P], F32R, name="ONES")
        nc.sync.dma_start(out=ONES, in_=ones[:, :])
        ONEC = ONES[:, 0:1]
        ONER = ONES[0:1, :]
        if causal:
            STAIR = pq.tile([P, 2 * SBLK], F32, name="STAIR")
            nc.sync.dma_start(out=STAIR, in_=stair[:, :])

        # ---- phase A+B: projections + RoPE (scoped pools) -------------
        with tc.tile_pool(name="pw", bufs=1) as pw, \
             tc.tile_pool(name="pht", bufs=6) as pht, \
             tc.tile_pool(name="ptmp", bufs=4) as ptmp, \
             tc.tile_pool(name="ps_p", bufs=8, space="PSUM") as ps_p:
            WQ = pw.tile([P, DCH, DQ], F32R, name="WQ")
            WK = pw.tile([P, DCH, HD], F32R, name="WK")
            WV = pw.tile([P, DCH, HD], F32R, name="WV")
            for c in range(DCH):
                nc.sync.dma_start(out=WQ[:, c, :], in_=wq[c * P:(c + 1) * P, :])
                nc.sync.dma_start(out=WK[:, c, :], in_=wk[c * P:(c + 1) * P, :])
                nc.sync.dma_start(out=WV[:, c, :], in_=wv[c * P:(c + 1) * P, :])
            COS = pw.tile([P, NKCH, S], F32, name="COS")
            SIN = pw.tile([P, NKCH, S], F32, name="SIN")
            for c in range(NKCH):
                nc.sync.dma_start(out=COS[:, c, :], in_=cosT[c * P:(c + 1) * P, :])
                nc.sync.dma_start(out=SIN[:, c, :], in_=sinT[c * P:(c + 1) * P, :])

            for sb in range(NSBLK):
                ssl = slice(sb * SBLK, (sb + 1) * SBLK)
                # main pass: QT / KT / V(si 0,1) -- 8 psum banks exactly,
                # V reuses the same ht tiles as QK
                psq = [ps_p.tile([P, SBLK], F32, name="pp") for _ in range(NQCH)]
                psk = [ps_p.tile([P, SBLK], F32, name="pp") for _ in range(NKCH)]
                psv01 = [ps_p.tile([P, SBLK], F32, name="pp") for _ in range(2)]
                for c in range(DCH):
                    ht = pht.tile([P, SBLK], F32R, name="ht")
                    nc.sync.dma_start(out=ht, in_=hT[c * P:(c + 1) * P, ssl])
                    for i in range(NQCH):
                        nc.tensor.matmul(psq[i], lhsT=WQ[:, c, i * P:(i + 1) * P],
                                         rhs=ht, start=(c == 0), stop=(c == DCH - 1))
                    for j in range(NKCH):
                        nc.tensor.matmul(psk[j], lhsT=WK[:, c, j * P:(j + 1) * P],
                                         rhs=ht, start=(c == 0), stop=(c == DCH - 1))
                    for si in range(2):
                        nc.tensor.matmul(psv01[si][:, :HD],
                                         lhsT=ht[:, si * P:(si + 1) * P],
                                         rhs=WV[:, c, :], start=(c == 0),
                                         stop=(c == DCH - 1))
                for si in range(2):
                    nc.vector.tensor_copy(VN[:, sb * (SBLK // P) + si, :],
                                          psv01[si][:, :HD])
                # RoPE drains (fused psum->SBUF)
                def rope_pair(p0, p1, out0, out1):
                    c0 = COS[:, 0, ssl]; c1 = COS[:, 1, ssl]
                    s0 = SIN[:, 0, ssl]; s1 = SIN[:, 1, ssl]
                    t1 = ptmp.tile([P, SBLK], F32, name="t")
                    t2 = ptmp.tile([P, SBLK], F32, name="t")
                    nc.vector.tensor_mul(t1, p0, c0)
                    nc.vector.tensor_mul(t2, p1, s0)
                    nc.vector.tensor_sub(out0, t1, t2)
                    t3 = ptmp.tile([P, SBLK], F32, name="t")
                    t4 = ptmp.tile([P, SBLK], F32, name="t")
                    nc.vector.tensor_mul(t3, p1, c1)
                    nc.vector.tensor_mul(t4, p0, s1)
                    nc.vector.tensor_add(out1, t3, t4)
                for h in range(HEADS_PER_CORE):
                    rope_pair(psq[2 * h], psq[2 * h + 1],
                              QT[:, 2 * h, ssl], QT[:, 2 * h + 1, ssl])
                rope_pair(psk[0], psk[1], KT[:, 0, ssl], KT[:, 1, ssl])

                # mini pass: V(si 2,3) -- re-reads the second half columns of ht
                psv23 = [ps_p.tile([P, SBLK], F32, name="pp") for _ in range(2)]
                for c in range(DCH):
                    htv = pht.tile([P, SBLK], F32R, name="ht")
                    nc.sync.dma_start(
                        out=htv[:, :HD],
                        in_=hT[c * P:(c + 1) * P,
                               sb * SBLK + 2 * P:sb * SBLK + 4 * P])
                    for si in range(2):
                        nc.tensor.matmul(psv23[si][:, :HD],
                                         lhsT=htv[:, si * P:(si + 1) * P],
                                         rhs=WV[:, c, :], start=(c == 0),
                                         stop=(c == DCH - 1))
                for si in range(2):
                    nc.vector.tensor_copy(VN[:, sb * (SBLK // P) + 2 + si, :],
                                          psv23[si][:, :HD])

        # ---- late persistent: o_proj weights + normalized outT --------
        patt = ctx.enter_context(tc.tile_pool(name="patt", bufs=1))
        WO = patt.tile([P, NQCH, D], F32R, name="WO")
        for c in range(NQCH):
            nc.sync.dma_start(out=WO[:, c, :], in_=wo[c * P:(c + 1) * P, :])
        OUTN = patt.tile([P, NQCH, S], F32R, name="OUTN")

        # ---- phase C: attention per head ------------------------------
        with tc.tile_pool(name="pexp", bufs=6) as pexp, \
             tc.tile_pool(name="pacc", bufs=2) as pacc, \
             tc.tile_pool(name="pmisc", bufs=2) as pmisc, \
             tc.tile_pool(name="pmask", bufs=4) as pmask, \
             tc.tile_pool(name="ps_s", bufs=2, space="PSUM") as ps_s, \
             tc.tile_pool(name="ps_o", bufs=4, space="PSUM") as ps_o, \
             tc.tile_pool(name="ps_r", bufs=2, space="PSUM") as ps_r:
            def emit_norm(pend):
                pso, acc, h, qb = pend
                qsl = slice(qb * SBLK, (qb + 1) * SBLK)
                pssum = ps_r.tile([P, SBLK], F32, name="pr")
                nc.tensor.matmul(pssum[0:1, :], lhsT=ONEC, rhs=acc)
                rsb = pmisc.tile([1, SBLK], F32R, name="rsb")
                with nc.allow_low_precision("f32r output is f32-width"):
                    nc.vector.reciprocal(rsb, pssum[0:1, :])
                psb = ps_r.tile([P, SBLK], F32, name="pr")
                nc.tensor.matmul(psb, lhsT=ONER, rhs=rsb)
                rbc = pmisc.tile([P, SBLK], F32R, name="rbc")
                nc.vector.tensor_copy(rbc, psb)
                for dvc in range(2):
                    nc.vector.tensor_mul(OUTN[:, 2 * h + dvc, qsl], pso[dvc], rbc)

            pending = None
            for h in range(HEADS_PER_CORE):
                for qb in range(NSBLK):
                    qsl = slice(qb * SBLK, (qb + 1) * SBLK)
                    klim = 4 * (qb + 1) if causal else NKC
                    pso = [ps_o.tile([P, SBLK], F32, name="pso") for _ in range(2)]
                    acc = pacc.tile([P, SBLK], F32R, name="acc")
                    for kc in range(klim):
                        pss = ps_s.tile([P, SBLK], F32, name="pss")
                        for c in range(NKCH):
                            nc.tensor.matmul(pss, lhsT=KT[:, c, kc * P:(kc + 1) * P],
                                             rhs=QT[:, 2 * h + c, qsl],
                                             start=(c == 0), stop=(c == NKCH - 1))
                        if causal and kc >= 4 * qb:
                            delta = 128 * kc - 512 * qb
                            nc.vector.tensor_add(pss, pss,
                                                 STAIR[:, 512 - delta:1024 - delta])
                        if not causal:
                            mt = pmask.tile([P, SBLK], F32, name="mt")
                            nc.sync.dma_start(
                                out=mt, in_=maskT[kc * P:(kc + 1) * P, qsl])
                            nc.vector.tensor_add(pss, pss, mt)
                        ex = pexp.tile([P, SBLK], F32R, name="ex")
                        nc.scalar.activation(ex, pss, EXP, scale=1.0 / 16.0)
                        if kc == 0:
                            nc.vector.tensor_copy(acc, ex)
                        else:
                            nc.vector.tensor_add(acc, acc, ex)
                        for dvc in range(2):
                            nc.tensor.matmul(pso[dvc],
                                             lhsT=VN[:, kc, dvc * P:(dvc + 1) * P],
                                             rhs=ex, start=(kc == 0),
                                             stop=(kc == klim - 1))
                    if pending is not None:
                        emit_norm(pending)
                    pending = (pso, acc, h, qb)
            emit_norm(pending)

        # ---- phase D: o_proj ------------------------------------------
        with tc.tile_pool(name="pfin", bufs=4) as pfin, \
             tc.tile_pool(name="ps_f", bufs=4, space="PSUM") as ps_f:
            for st in range(NKC):
                stsl = slice(st * P, (st + 1) * P)
                for nb in range(NSBLK):
                    psf = ps_f.tile([P, SBLK], F32, name="psf")
                    for dvc in range(NQCH):
                        nc.tensor.matmul(psf, lhsT=OUTN[:, dvc, stsl],
                                         rhs=WO[:, dvc, nb * SBLK:(nb + 1) * SBLK],
                                         start=(dvc == 0), stop=(dvc == NQCH - 1))
                    fsb = pfin.tile([P, SBLK], F32, name="fsb")
                    nc.scalar.copy(fsb, psf)
                    nc.sync.dma_start(out=outp[stsl, nb * SBLK:(nb + 1) * SBLK], in_=fsb)

    nc.finalize()
    return nc


def _get_nc(causal: bool):
    key = bool(causal)
    if key not in _BUILD_CACHE:
        _BUILD_CACHE[key] = _build(causal)
    return _BUILD_CACHE[key]


def _rope_tables(position_ids_b):
    # cosT/sinT: [HD, S] fp32, transposed layout for the [d, s] dataflow
    pos = np.asarray(position_ids_b, dtype=np.float64)
    inv = 1.0 / (ROPE_BASE ** (np.arange(0, HD, 2, dtype=np.float64) / HD))
    f = pos[:, None] * inv[None, :]            # [S, HD/2]
    emb = np.concatenate([f, f], axis=1)       # [S, HD]
    cosT = np.ascontiguousarray(np.cos(emb).T.astype(np.float32))
    sinT = np.ascontiguousarray(np.sin(emb).T.astype(np.float32))
    return cosT, sinT


def _is_causal(attention_mask):
    m = np.asarray(attention_mask)
    if m.shape != (B, 1, S, S):
        return False
    tri = np.tril(np.ones((S, S), dtype=bool))
    canon = np.where(tri, np.float32(0.0), np.float32(-1e9))
    return all(np.array_equal(m[b, 0], canon) for b in range(B))


_ONES_NP = np.ones((P, P), dtype=np.float32)


def _stair():
    # stair[p, j] = 0 if (j - 512) >= p else NEG_BIG   (width 1024)
    j = np.arange(2 * SBLK)[None, :] - SBLK
    p = np.arange(P)[:, None]
    return np.where(j >= p, np.float32(0.0), np.float32(NEG_BIG)).astype(np.float32)


def kernel(hidden_state, attention_mask, position_ids, Wq, Wk, Wv, Wo,
           _trace=False, _tmpdir=None):
    global LAST_EXEC_TIME_NS
    hidden_state = np.asarray(hidden_state, dtype=np.float32)
    Wq = np.asarray(Wq, dtype=np.float32)
    Wk = np.asarray(Wk, dtype=np.float32)
    Wv = np.asarray(Wv, dtype=np.float32)
    Wo = np.asarray(Wo, dtype=np.float32)

    causal = _is_causal(attention_mask)
    nc = _get_nc(causal)

    stair = _stair() if causal else None
    in_maps = []
    per_batch = {}
    for b in range(B):
        hTb = np.ascontiguousarray(hidden_state[b].T)          # [D, S]
        cosT, sinT = _rope_tables(position_ids[b])
        mb = None
        if not causal:
            mb = np.ascontiguousarray(
                np.asarray(attention_mask, dtype=np.float32)[b, 0].T * 16.0)
        per_batch[b] = (hTb, cosT, sinT, mb)

    for core in range(8):
        b = core // 4
        hp = core % 4
        hTb, cosT, sinT, mb = per_batch[b]
        im = {
            "hT": hTb,
            "ones": _ONES_NP,
            "wq": np.ascontiguousarray(Wq[:, hp * DQ:(hp + 1) * DQ]),
            "wk": Wk,
            "wv": Wv,
            "wo": np.ascontiguousarray(Wo[hp * DQ:(hp + 1) * DQ, :]),
            "cosT": cosT,
            "sinT": sinT,
        }
        if causal:
            im["stair"] = stair
        else:
            im["maskT16"] = mb
        in_maps.append(im)

    res = run_bass_kernel_spmd(nc, in_maps, core_ids=list(range(8)),
                               trace=_trace, tmpdir=_tmpdir)
    LAST_EXEC_TIME_NS = res.exec_time_ns

    out = np.empty((B, S, D), dtype=np.float32)
    for b in range(B):
        acc = res.results[4 * b]["out_partial"].astype(np.float32).copy()
        for hp in range(1, 4):
            acc += res.results[4 * b + hp]["out_partial"]
        out[b] = acc
    return out


# revision 13
# speedup vs baseline: 1.0394x; 1.0394x over previous
"""GemmaAttention (B=2, S=2048, D=2048, H=8, KV=1, HD=256) on 8 trn2 NeuronCores.

Sharding: DP=2 over batch x TP=4 over head-pairs. Core c handles batch c//4 and
heads {2*(c%4), 2*(c%4)+1}. Each core computes its partial o_proj output
(row-parallel Wo); the host sums the 4 partials per batch (the all-reduce is
folded into the host-side unshard).

Dataflow on each core (everything float32r on the PE at full rate):
  QT[dq,s]  = Wq_sl.T @ hT   (hT = hidden[b].T, host-transposed)
  KT[dk,s]  = Wk.T   @ hT
  V[s,dv]   = (hT chunks as lhsT) @ Wv
  RoPE applied to QT/KT in the psum->SBUF drain (DVE), with 1/sqrt(HD) folded
  into the exp's scale argument.
  scoresT[k,q] = KT_chunk.T @ QT  (per head)
  expT = ACT Exp(scoresT * 1/16) (+ causal staircase / external mask)
  outT[dv,q] += V_chunk.T @ expT ; denominators via DVE accumulation of expT
  plus a ones-vector matmul partition-reduce; normalize outT by 1/sum.
  out_partial[s,:] = outTn_chunk.T @ Wo_sl   -> DMA to DRAM.
"""

import numpy as np

import concourse.bass as bass
import concourse.tile as tile
import concourse.mybir as mybir
from concourse import bacc
from concourse.bass_utils import run_bass_kernel_spmd
from concourse._compat import with_exitstack  # noqa: F401

P = 128
B, S, D = 2, 2048, 2048
H, KV, HD = 8, 1, 256
ROPE_BASE = 10000.0
NEG_BIG = -1.0e30

HEADS_PER_CORE = 2
DQ = HEADS_PER_CORE * HD          # 512 q-dims per core
DCH = D // P                      # 16 contraction chunks
SBLK = 512                        # s-tile for projection rhs / q-tile
NSBLK = S // SBLK                 # 4
NKC = S // P                      # 16 key chunks
NQCH = DQ // P                    # 4 QT partition chunks
NKCH = HD // P                    # 2 KT partition chunks

F32 = mybir.dt.float32
F32R = mybir.dt.float32r
EXP = mybir.ActivationFunctionType.Exp

# exec time of the last traced run (set by run_spmd when tracing)
LAST_EXEC_TIME_NS = None

_BUILD_CACHE = {}


def _build(causal: bool):
    nc = bacc.Bacc()

    hT = nc.declare_dram_parameter("hT", [D, S], F32R, isOutput=False)
    wq = nc.declare_dram_parameter("wq", [D, DQ], F32R, isOutput=False)
    wk = nc.declare_dram_parameter("wk", [D, HD], F32R, isOutput=False)
    wv = nc.declare_dram_parameter("wv", [D, HD], F32R, isOutput=False)
    wo = nc.declare_dram_parameter("wo", [DQ, D], F32R, isOutput=False)
    cosT = nc.declare_dram_parameter("cosT", [HD, S], F32, isOutput=False)
    sinT = nc.declare_dram_parameter("sinT", [HD, S], F32, isOutput=False)
    ones = nc.declare_dram_parameter("ones", [P, P], F32R, isOutput=False)
    if causal:
        stair = nc.declare_dram_parameter("stair", [P, 2 * SBLK], F32, isOutput=False)
    else:
        maskT = nc.declare_dram_parameter("maskT16", [S, S], F32, isOutput=False)
    outp = nc.declare_dram_parameter("out_partial", [S, D], F32, isOutput=True)

    from contextlib import ExitStack
    with tile.TileContext(nc) as tc, ExitStack() as ctx:
        # persistent across phases: QT/KT/V + small consts
        pq = ctx.enter_context(tc.tile_pool(name="pq", bufs=1))
        QT = pq.tile([P, NQCH, S], F32R, name="QT")
        KT = pq.tile([P, NKCH, S], F32R, name="KT")
        VN = pq.tile([P, NKC, HD], F32R, name="VN")
        ONES = pq.tile([P, P], F32R, name="ONES")
        nc.sync.dma_start(out=ONES, in_=ones[:, :])
        ONEC = ONES[:, 0:1]
        ONER = ONES[0:1, :]
        if causal:
            STAIR = pq.tile([P, 2 * SBLK], F32, name="STAIR")
            nc.sync.dma_start(out=STAIR, in_=stair[:, :])

        # ---- phase A+B: projections + RoPE (scoped pools) -------------
        with tc.tile_pool(name="pw", bufs=1) as pw, \
             tc.tile_pool(name="pht", bufs=6) as pht, \
             tc.tile_pool(name="ptmp", bufs=4) as ptmp, \
             tc.tile_pool(name="ps_p", bufs=8, space="PSUM") as ps_p:
            WQ = pw.tile([P, DCH, DQ], F32R, name="WQ")
            WK = pw.tile([P, DCH, HD], F32R, name="WK")
            WV = pw.tile([P, DCH, HD], F32R, name="WV")
            COS = pw.tile([P, NKCH, S], F32, name="COS")
            SIN = pw.tile([P, NKCH, S], F32, name="SIN")

            for sb in range(NSBLK):
                ssl = slice(sb * SBLK, (sb + 1) * SBLK)
                # main pass: QT / KT / V(si 0,1) -- 8 psum banks exactly,
                # V reuses the same ht tiles as QK
                psq = [ps_p.tile([P, SBLK], F32, name="pp") for _ in range(NQCH)]
                psk = [ps_p.tile([P, SBLK], F32, name="pp") for _ in range(NKCH)]
                psv01 = [ps_p.tile([P, SBLK], F32, name="pp") for _ in range(2)]
                for c in range(DCH):
                    if sb == 0:
                        # weight chunks stream just ahead of their first use
                        nc.sync.dma_start(out=WQ[:, c, :], in_=wq[c * P:(c + 1) * P, :])
                        nc.sync.dma_start(out=WK[:, c, :], in_=wk[c * P:(c + 1) * P, :])
                        nc.sync.dma_start(out=WV[:, c, :], in_=wv[c * P:(c + 1) * P, :])
                        if c < NKCH:
                            nc.sync.dma_start(out=COS[:, c, :],
                                              in_=cosT[c * P:(c + 1) * P, :])
                            nc.sync.dma_start(out=SIN[:, c, :],
                                              in_=sinT[c * P:(c + 1) * P, :])
                    ht = pht.tile([P, SBLK], F32R, name="ht")
                    nc.sync.dma_start(out=ht, in_=hT[c * P:(c + 1) * P, ssl])
                    for i in range(NQCH):
                        nc.tensor.matmul(psq[i], lhsT=WQ[:, c, i * P:(i + 1) * P],
                                         rhs=ht, start=(c == 0), stop=(c == DCH - 1))
                    for j in range(NKCH):
                        nc.tensor.matmul(psk[j], lhsT=WK[:, c, j * P:(j + 1) * P],
                                         rhs=ht, start=(c == 0), stop=(c == DCH - 1))
                    for si in range(2):
                        nc.tensor.matmul(psv01[si][:, :HD],
                                         lhsT=ht[:, si * P:(si + 1) * P],
                                         rhs=WV[:, c, :], start=(c == 0),
                                         stop=(c == DCH - 1))
                for si in range(2):
                    nc.vector.tensor_copy(VN[:, sb * (SBLK // P) + si, :],
                                          psv01[si][:, :HD])
                # RoPE drains (fused psum->SBUF)
                def rope_pair(p0, p1, out0, out1):
                    c0 = COS[:, 0, ssl]; c1 = COS[:, 1, ssl]
                    s0 = SIN[:, 0, ssl]; s1 = SIN[:, 1, ssl]
                    t1 = ptmp.tile([P, SBLK], F32, name="t")
                    t2 = ptmp.tile([P, SBLK], F32, name="t")
                    nc.vector.tensor_mul(t1, p0, c0)
                    nc.vector.tensor_mul(t2, p1, s0)
                    nc.vector.tensor_sub(out0, t1, t2)
                    t3 = ptmp.tile([P, SBLK], F32, name="t")
                    t4 = ptmp.tile([P, SBLK], F32, name="t")
                    nc.vector.tensor_mul(t3, p1, c1)
                    nc.vector.tensor_mul(t4, p0, s1)
                    nc.vector.tensor_add(out1, t3, t4)
                for h in range(HEADS_PER_CORE):
                    rope_pair(psq[2 * h], psq[2 * h + 1],
                              QT[:, 2 * h, ssl], QT[:, 2 * h + 1, ssl])
                rope_pair(psk[0], psk[1], KT[:, 0, ssl], KT[:, 1, ssl])

                # mini pass: V(si 2,3) -- re-reads the second half columns of ht
                psv23 = [ps_p.tile([P, SBLK], F32, name="pp") for _ in range(2)]
                for c in range(DCH):
                    htv = pht.tile([P, SBLK], F32R, name="ht")
                    nc.sync.dma_start(
                        out=htv[:, :HD],
                        in_=hT[c * P:(c + 1) * P,
                               sb * SBLK + 2 * P:sb * SBLK + 4 * P])
                    for si in range(2):
                        nc.tensor.matmul(psv23[si][:, :HD],
                                         lhsT=htv[:, si * P:(si + 1) * P],
                                         rhs=WV[:, c, :], start=(c == 0),
                                         stop=(c == DCH - 1))
                for si in range(2):
                    nc.vector.tensor_copy(VN[:, sb * (SBLK // P) + 2 + si, :],
                                          psv23[si][:, :HD])

        # ---- late persistent: o_proj weights + normalized outT --------
        patt = ctx.enter_context(tc.tile_pool(name="patt", bufs=1))
        WO = patt.tile([P, NQCH, D], F32R, name="WO")
        for c in range(NQCH):
            nc.sync.dma_start(out=WO[:, c, :], in_=wo[c * P:(c + 1) * P, :])
        OUTN = patt.tile([P, NQCH, S], F32R, name="OUTN")

        # ---- phase C: attention per head ------------------------------
        with tc.tile_pool(name="pexp", bufs=6) as pexp, \
             tc.tile_pool(name="pacc", bufs=4) as pacc, tc.tile_pool(name="pou", bufs=8) as pou, \
             tc.tile_pool(name="pmisc", bufs=2) as pmisc, \
             tc.tile_pool(name="pmask", bufs=4) as pmask, \
             tc.tile_pool(name="ps_s", bufs=2, space="PSUM") as ps_s, \
             tc.tile_pool(name="ps_o", bufs=4, space="PSUM") as ps_o, \
             tc.tile_pool(name="ps_r", bufs=2, space="PSUM") as ps_r:
            def emit_norm(pend):
                ou, acc, h, qb = pend
                qsl = slice(qb * SBLK, (qb + 1) * SBLK)
                pssum = ps_r.tile([P, SBLK], F32, name="pr")
                nc.tensor.matmul(pssum[0:1, :], lhsT=ONEC, rhs=acc)
                rsb = pmisc.tile([1, SBLK], F32R, name="rsb")
                with nc.allow_low_precision("f32r output is f32-width"):
                    nc.vector.reciprocal(rsb, pssum[0:1, :])
                psb = ps_r.tile([P, SBLK], F32, name="pr")
                nc.tensor.matmul(psb, lhsT=ONER, rhs=rsb)
                rbc = pmisc.tile([P, SBLK], F32R, name="rbc")
                nc.vector.tensor_copy(rbc, psb)
                for dvc in range(2):
                    nc.vector.tensor_mul(OUTN[:, 2 * h + dvc, qsl], ou[dvc], rbc)

            from collections import deque
            pending = deque()
            for h in range(HEADS_PER_CORE):
                for qb in range(NSBLK):
                    qsl = slice(qb * SBLK, (qb + 1) * SBLK)
                    klim = 4 * (qb + 1) if causal else NKC
                    pso = [ps_o.tile([P, SBLK], F32, name="pso") for _ in range(2)]
                    acc = pacc.tile([P, SBLK], F32R, name="acc")
                    for kc in range(klim):
                        pss = ps_s.tile([P, SBLK], F32, name="pss")
                        for c in range(NKCH):
                            nc.tensor.matmul(pss, lhsT=KT[:, c, kc * P:(kc + 1) * P],
                                             rhs=QT[:, 2 * h + c, qsl],
                                             start=(c == 0), stop=(c == NKCH - 1))
                        if causal and kc >= 4 * qb:
                            delta = 128 * kc - 512 * qb
                            nc.vector.tensor_add(pss, pss,
                                                 STAIR[:, 512 - delta:1024 - delta])
                        if not causal:
                            mt = pmask.tile([P, SBLK], F32, name="mt")
                            nc.sync.dma_start(
                                out=mt, in_=maskT[kc * P:(kc + 1) * P, qsl])
                            nc.vector.tensor_add(pss, pss, mt)
                        ex = pexp.tile([P, SBLK], F32R, name="ex")
                        nc.scalar.activation(ex, pss, EXP, scale=1.0 / 16.0)
                        if kc == 0:
                            nc.vector.tensor_copy(acc, ex)
                        else:
                            nc.vector.tensor_add(acc, acc, ex)
                        for dvc in range(2):
                            nc.tensor.matmul(pso[dvc],
                                             lhsT=VN[:, kc, dvc * P:(dvc + 1) * P],
                                             rhs=ex, start=(kc == 0),
                                             stop=(kc == klim - 1))
                    # drain attn@V unnormalized so the psum slots free early
                    ou = [pou.tile([P, SBLK], F32R, name="ou") for _ in range(2)]
                    for dvc in range(2):
                        nc.vector.tensor_copy(ou[dvc], pso[dvc])
                    pending.append((ou, acc, h, qb))
                    if len(pending) > 2:
                        emit_norm(pending.popleft())
            while pending:
                emit_norm(pending.popleft())

        # ---- phase D: o_proj ------------------------------------------
        with tc.tile_pool(name="pfin", bufs=4) as pfin, \
             tc.tile_pool(name="ps_f", bufs=4, space="PSUM") as ps_f:
            for st in range(NKC):
                stsl = slice(st * P, (st + 1) * P)
                for nb in range(NSBLK):
                    psf = ps_f.tile([P, SBLK], F32, name="psf")
                    for dvc in range(NQCH):
                        nc.tensor.matmul(psf, lhsT=OUTN[:, dvc, stsl],
                                         rhs=WO[:, dvc, nb * SBLK:(nb + 1) * SBLK],
                                         start=(dvc == 0), stop=(dvc == NQCH - 1))
                    fsb = pfin.tile([P, SBLK], F32, name="fsb")
                    nc.scalar.copy(fsb, psf)
                    nc.sync.dma_start(out=outp[stsl, nb * SBLK:(nb + 1) * SBLK], in_=fsb)

    nc.finalize()
    return nc


def _get_nc(causal: bool):
    key = bool(causal)
    if key not in _BUILD_CACHE:
        _BUILD_CACHE[key] = _build(causal)
    return _BUILD_CACHE[key]


def _rope_tables(position_ids_b):
    # cosT/sinT: [HD, S] fp32, transposed layout for the [d, s] dataflow
    pos = np.asarray(position_ids_b, dtype=np.float64)
    inv = 1.0 / (ROPE_BASE ** (np.arange(0, HD, 2, dtype=np.float64) / HD))
    f = pos[:, None] * inv[None, :]            # [S, HD/2]
    emb = np.concatenate([f, f], axis=1)       # [S, HD]
    cosT = np.ascontiguousarray(np.cos(emb).T.astype(np.float32))
    sinT = np.ascontiguousarray(np.sin(emb).T.astype(np.float32))
    return cosT, sinT


def _is_causal(attention_mask):
    m = np.asarray(attention_mask)
    if m.shape != (B, 1, S, S):
        return False
    tri = np.tril(np.ones((S, S), dtype=bool))
    canon = np.where(tri, np.float32(0.0), np.float32(-1e9))
    return all(np.array_equal(m[b, 0], canon) for b in range(B))


_ONES_NP = np.ones((P, P), dtype=np.float32)


def _stair():
    # stair[p, j] = 0 if (j - 512) >= p else NEG_BIG   (width 1024)
    j = np.arange(2 * SBLK)[None, :] - SBLK
    p = np.arange(P)[:, None]
    return np.where(j >= p, np.float32(0.0), np.float32(NEG_BIG)).astype(np.float32)


def kernel(hidden_state, attention_mask, position_ids, Wq, Wk, Wv, Wo,
           _trace=False, _tmpdir=None):
    global LAST_EXEC_TIME_NS
    hidden_state = np.asarray(hidden_state, dtype=np.float32)
    Wq = np.asarray(Wq, dtype=np.float32)
    Wk = np.asarray(Wk, dtype=np.float32)
    Wv = np.asarray(Wv, dtype=np.float32)
    Wo = np.asarray(Wo, dtype=np.float32)

    causal = _is_causal(attention_mask)
    nc = _get_nc(causal)

    stair = _stair() if causal else None
    in_maps = []
    per_batch = {}
    for b in range(B):
        hTb = np.ascontiguousarray(hidden_state[b].T)          # [D, S]
        cosT, sinT = _rope_tables(position_ids[b])
        mb = None
        if not causal:
            mb = np.ascontiguousarray(
                np.asarray(attention_mask, dtype=np.float32)[b, 0].T * 16.0)
        per_batch[b] = (hTb, cosT, sinT, mb)

    for core in range(8):
        b = core // 4
        hp = core % 4
        hTb, cosT, sinT, mb = per_batch[b]
        im = {
            "hT": hTb,
            "ones": _ONES_NP,
            "wq": np.ascontiguousarray(Wq[:, hp * DQ:(hp + 1) * DQ]),
            "wk": Wk,
            "wv": Wv,
            "wo": np.ascontiguousarray(Wo[hp * DQ:(hp + 1) * DQ, :]),
            "cosT": cosT,
            "sinT": sinT,
        }
        if causal:
            im["stair"] = stair
        else:
            im["maskT16"] = mb
        in_maps.append(im)

    res = run_bass_kernel_spmd(nc, in_maps, core_ids=list(range(8)),
                               trace=_trace, tmpdir=_tmpdir)
    LAST_EXEC_TIME_NS = res.exec_time_ns

    out = np.empty((B, S, D), dtype=np.float32)
    for b in range(B):
        acc = res.results[4 * b]["out_partial"].astype(np.float32).copy()
        for hp in range(1, 4):
            acc += res.results[4 * b + hp]["out_partial"]
        out[b] = acc
    return out


# revision 15
# speedup vs baseline: 1.0861x; 1.0449x over previous
"""GemmaAttention (B=2, S=2048, D=2048, H=8, KV=1, HD=256) on 8 trn2 NeuronCores.

Sharding: DP=2 over batch x TP=4 over head-pairs. Core c handles batch c//4 and
heads {2*(c%4), 2*(c%4)+1}. Each core computes its partial o_proj output
(row-parallel Wo); the host sums the 4 partials per batch (the all-reduce is
folded into the host-side unshard).

Dataflow on each core (everything float32r on the PE at full rate):
  QT[dq,s]  = Wq_sl.T @ hT   (hT = hidden[b].T, host-transposed)
  KT[dk,s]  = Wk.T   @ hT
  V[s,dv]   = (hT chunks as lhsT) @ Wv
  RoPE applied to QT/KT in the psum->SBUF drain (DVE), with 1/sqrt(HD) folded
  into the exp's scale argument.
  scoresT[k,q] = KT_chunk.T @ QT  (per head)
  expT = ACT Exp(scoresT * 1/16) (+ causal staircase / external mask)
  outT[dv,q] += V_chunk.T @ expT ; denominators via DVE accumulation of expT
  plus a ones-vector matmul partition-reduce; normalize outT by 1/sum.
  out_partial[s,:] = outTn_chunk.T @ Wo_sl   -> DMA to DRAM.
"""

import numpy as np

import concourse.bass as bass
import concourse.tile as tile
import concourse.mybir as mybir
from concourse import bacc
from concourse.bass_utils import run_bass_kernel_spmd
from concourse._compat import with_exitstack  # noqa: F401

P = 128
B, S, D = 2, 2048, 2048
H, KV, HD = 8, 1, 256
ROPE_BASE = 10000.0
NEG_BIG = -1.0e30

HEADS_PER_CORE = 2
DQ = HEADS_PER_CORE * HD          # 512 q-dims per core
DCH = D // P                      # 16 contraction chunks
SBLK = 512                        # s-tile for projection rhs / q-tile
NSBLK = S // SBLK                 # 4
NKC = S // P                      # 16 key chunks
NQCH = DQ // P                    # 4 QT partition chunks
NKCH = HD // P                    # 2 KT partition chunks

F32 = mybir.dt.float32
F32R = mybir.dt.float32r
EXP = mybir.ActivationFunctionType.Exp

# exec time of the last traced run (set by run_spmd when tracing)
LAST_EXEC_TIME_NS = None

_BUILD_CACHE = {}


def _build(causal: bool):
    nc = bacc.Bacc()

    hT = nc.declare_dram_parameter("hT", [D, S], F32R, isOutput=False)
    wq = nc.declare_dram_parameter("wq", [D, DQ], F32R, isOutput=False)
    wk = nc.declare_dram_parameter("wk", [D, HD], F32R, isOutput=False)
    wv = nc.declare_dram_parameter("wv", [D, HD], F32R, isOutput=False)
    wo = nc.declare_dram_parameter("wo", [DQ, D], F32R, isOutput=False)
    cosT = nc.declare_dram_parameter("cosT", [HD, S], F32, isOutput=False)
    sinT = nc.declare_dram_parameter("sinT", [HD, S], F32, isOutput=False)
    ones = nc.declare_dram_parameter("ones", [P, P], F32R, isOutput=False)
    if causal:
        stair = nc.declare_dram_parameter("stair", [P, 2 * SBLK], F32, isOutput=False)
    else:
        maskT = nc.declare_dram_parameter("emaskT", [S, S], F32, isOutput=False)
    outp = nc.declare_dram_parameter("out_partial", [S, D], F32, isOutput=True)

    from contextlib import ExitStack
    with tile.TileContext(nc) as tc, ExitStack() as ctx:
        # persistent across phases: QT/KT/V + small consts
        pq = ctx.enter_context(tc.tile_pool(name="pq", bufs=1))
        QT = pq.tile([P, NQCH, S], F32R, name="QT")
        KT = pq.tile([P, NKCH, S], F32R, name="KT")
        VN = pq.tile([P, NKC, HD], F32R, name="VN")
        ONES = pq.tile([P, P], F32R, name="ONES")
        nc.sync.dma_start(out=ONES, in_=ones[:, :])
        ONEC = ONES[:, 0:1]
        ONER = ONES[0:1, :]
        if causal:
            STAIR = pq.tile([P, 2 * SBLK], F32, name="STAIR")
            nc.sync.dma_start(out=STAIR, in_=stair[:, :])

        # ---- phase A+B: projections + RoPE (scoped pools) -------------
        with tc.tile_pool(name="pw", bufs=1) as pw, \
             tc.tile_pool(name="pht", bufs=6) as pht, \
             tc.tile_pool(name="ptmp", bufs=4) as ptmp, \
             tc.tile_pool(name="ps_p", bufs=8, space="PSUM") as ps_p:
            WQ = pw.tile([P, DCH, DQ], F32R, name="WQ")
            WK = pw.tile([P, DCH, HD], F32R, name="WK")
            WV = pw.tile([P, DCH, HD], F32R, name="WV")
            COS = pw.tile([P, NKCH, S], F32, name="COS")
            SIN = pw.tile([P, NKCH, S], F32, name="SIN")

            for sb in range(NSBLK):
                ssl = slice(sb * SBLK, (sb + 1) * SBLK)
                # main pass: QT / KT / V(si 0,1) -- 8 psum banks exactly,
                # V reuses the same ht tiles as QK
                psq = [ps_p.tile([P, SBLK], F32, name="pp") for _ in range(NQCH)]
                psk = [ps_p.tile([P, SBLK], F32, name="pp") for _ in range(NKCH)]
                psv01 = [ps_p.tile([P, SBLK], F32, name="pp") for _ in range(2)]
                for c in range(DCH):
                    if sb == 0:
                        # weight chunks stream just ahead of their first use
                        nc.sync.dma_start(out=WQ[:, c, :], in_=wq[c * P:(c + 1) * P, :])
                        nc.sync.dma_start(out=WK[:, c, :], in_=wk[c * P:(c + 1) * P, :])
                        nc.sync.dma_start(out=WV[:, c, :], in_=wv[c * P:(c + 1) * P, :])
                        if 2 <= c < 2 + NKCH:
                            nc.sync.dma_start(out=COS[:, c - 2, :],
                                              in_=cosT[(c - 2) * P:(c - 1) * P, :])
                            nc.sync.dma_start(out=SIN[:, c - 2, :],
                                              in_=sinT[(c - 2) * P:(c - 1) * P, :])
                    ht = pht.tile([P, SBLK], F32R, name="ht")
                    nc.sync.dma_start(out=ht, in_=hT[c * P:(c + 1) * P, ssl])
                    for i in range(NQCH):
                        nc.tensor.matmul(psq[i], lhsT=WQ[:, c, i * P:(i + 1) * P],
                                         rhs=ht, start=(c == 0), stop=(c == DCH - 1))
                    for j in range(NKCH):
                        nc.tensor.matmul(psk[j], lhsT=WK[:, c, j * P:(j + 1) * P],
                                         rhs=ht, start=(c == 0), stop=(c == DCH - 1))
                    for si in range(2):
                        nc.tensor.matmul(psv01[si][:, :HD],
                                         lhsT=ht[:, si * P:(si + 1) * P],
                                         rhs=WV[:, c, :], start=(c == 0),
                                         stop=(c == DCH - 1))
                for si in range(2):
                    nc.vector.tensor_copy(VN[:, sb * (SBLK // P) + si, :],
                                          psv01[si][:, :HD])
                # RoPE drains (fused psum->SBUF)
                def rope_pair(p0, p1, out0, out1):
                    c0 = COS[:, 0, ssl]; c1 = COS[:, 1, ssl]
                    s0 = SIN[:, 0, ssl]; s1 = SIN[:, 1, ssl]
                    t1 = ptmp.tile([P, SBLK], F32, name="t")
                    t2 = ptmp.tile([P, SBLK], F32, name="t")
                    nc.vector.tensor_mul(t1, p0, c0)
                    nc.vector.tensor_mul(t2, p1, s0)
                    nc.vector.tensor_sub(out0, t1, t2)
                    t3 = ptmp.tile([P, SBLK], F32, name="t")
                    t4 = ptmp.tile([P, SBLK], F32, name="t")
                    nc.vector.tensor_mul(t3, p1, c1)
                    nc.vector.tensor_mul(t4, p0, s1)
                    nc.vector.tensor_add(out1, t3, t4)
                for h in range(HEADS_PER_CORE):
                    rope_pair(psq[2 * h], psq[2 * h + 1],
                              QT[:, 2 * h, ssl], QT[:, 2 * h + 1, ssl])
                rope_pair(psk[0], psk[1], KT[:, 0, ssl], KT[:, 1, ssl])

                # mini pass: V(si 2,3) -- re-reads the second half columns of ht
                psv23 = [ps_p.tile([P, SBLK], F32, name="pp") for _ in range(2)]
                for c in range(DCH):
                    htv = pht.tile([P, SBLK], F32R, name="ht")
                    nc.sync.dma_start(
                        out=htv[:, :HD],
                        in_=hT[c * P:(c + 1) * P,
                               sb * SBLK + 2 * P:sb * SBLK + 4 * P])
                    for si in range(2):
                        nc.tensor.matmul(psv23[si][:, :HD],
                                         lhsT=htv[:, si * P:(si + 1) * P],
                                         rhs=WV[:, c, :], start=(c == 0),
                                         stop=(c == DCH - 1))
                for si in range(2):
                    nc.vector.tensor_copy(VN[:, sb * (SBLK // P) + 2 + si, :],
                                          psv23[si][:, :HD])

        # ---- late persistent: o_proj weights + normalized outT --------
        patt = ctx.enter_context(tc.tile_pool(name="patt", bufs=1))
        WO = patt.tile([P, NQCH, D], F32R, name="WO")
        for c in range(NQCH):
            nc.sync.dma_start(out=WO[:, c, :], in_=wo[c * P:(c + 1) * P, :])
        OUTN = patt.tile([P, NQCH, S], F32R, name="OUTN")

        # ---- phase C+D: attention + interleaved o_proj ----------------
        with tc.tile_pool(name="pexp", bufs=6) as pexp, \
             tc.tile_pool(name="pacc", bufs=4) as pacc, \
             tc.tile_pool(name="pou", bufs=8) as pou, \
             tc.tile_pool(name="pmisc", bufs=2) as pmisc, \
             tc.tile_pool(name="pmask", bufs=4) as pmask, \
             tc.tile_pool(name="pfin", bufs=3) as pfin, \
             tc.tile_pool(name="ps_s", bufs=3, space="PSUM") as ps_s, \
             tc.tile_pool(name="ps_o", bufs=2, space="PSUM") as ps_o, \
             tc.tile_pool(name="ps_r", bufs=1, space="PSUM") as ps_r, \
             tc.tile_pool(name="ps_f", bufs=2, space="PSUM") as ps_f:

            def emit_norm(pend):
                ou, acc, h, qb = pend
                qsl = slice(qb * SBLK, (qb + 1) * SBLK)
                pssum = ps_r.tile([P, SBLK], F32, name="pr")
                nc.tensor.matmul(pssum[0:1, :], lhsT=ONEC, rhs=acc)
                rsb = pmisc.tile([1, SBLK], F32R, name="rsb")
                with nc.allow_low_precision("f32r output is f32-width"):
                    nc.vector.reciprocal(rsb, pssum[0:1, :])
                psb = ps_r.tile([P, SBLK], F32, name="pr")
                nc.tensor.matmul(psb, lhsT=ONER, rhs=rsb)
                rbc = pmisc.tile([P, SBLK], F32R, name="rbc")
                nc.vector.tensor_copy(rbc, psb)
                for dvc in range(2):
                    nc.vector.tensor_mul(OUTN[:, 2 * h + dvc, qsl], ou[dvc], rbc)

            def emit_oproj_quarter(qb):
                # output rows s in [qb*512, (qb+1)*512) need OUTN[:, :, rows]
                for st in range(4 * qb, 4 * qb + 4):
                    stsl = slice(st * P, (st + 1) * P)
                    for nb in range(NSBLK):
                        psf = ps_f.tile([P, SBLK], F32, name="psf")
                        for dvc in range(NQCH):
                            nc.tensor.matmul(psf, lhsT=OUTN[:, dvc, stsl],
                                             rhs=WO[:, dvc, nb * SBLK:(nb + 1) * SBLK],
                                             start=(dvc == 0), stop=(dvc == NQCH - 1))
                        fsb = pfin.tile([P, SBLK], F32, name="fsb")
                        nc.scalar.copy(fsb, psf)
                        nc.sync.dma_start(out=outp[stsl, nb * SBLK:(nb + 1) * SBLK],
                                          in_=fsb)

            from collections import deque
            pending = deque()
            for h in range(HEADS_PER_CORE):
                for qb in range(NSBLK):
                    qsl = slice(qb * SBLK, (qb + 1) * SBLK)
                    klim = 4 * (qb + 1) if causal else NKC
                    pso = [ps_o.tile([P, SBLK], F32, name="pso") for _ in range(2)]
                    acc = pacc.tile([P, SBLK], F32R, name="acc")
                    for kc in range(klim):
                        pss = ps_s.tile([P, SBLK], F32, name="pss")
                        for c in range(NKCH):
                            nc.tensor.matmul(pss, lhsT=KT[:, c, kc * P:(kc + 1) * P],
                                             rhs=QT[:, 2 * h + c, qsl],
                                             start=(c == 0), stop=(c == NKCH - 1))
                        ex = pexp.tile([P, SBLK], F32R, name="ex")
                        nc.scalar.activation(ex, pss, EXP, scale=1.0 / 16.0)
                        if causal and kc >= 4 * qb:
                            delta = 128 * kc - 512 * qb
                            nc.vector.tensor_mul(ex, ex,
                                                 STAIR[:, 512 - delta:1024 - delta])
                        if not causal:
                            mt = pmask.tile([P, SBLK], F32, name="mt")
                            nc.sync.dma_start(
                                out=mt, in_=maskT[kc * P:(kc + 1) * P, qsl])
                            nc.vector.tensor_mul(ex, ex, mt)
                        if kc == 0:
                            nc.vector.tensor_copy(acc, ex)
                        else:
                            nc.vector.tensor_add(acc, acc, ex)
                        for dvc in range(2):
                            nc.tensor.matmul(pso[dvc],
                                             lhsT=VN[:, kc, dvc * P:(dvc + 1) * P],
                                             rhs=ex, start=(kc == 0),
                                             stop=(kc == klim - 1))
                    # drain attn@V unnormalized so the psum slots free early
                    ou = [pou.tile([P, SBLK], F32R, name="ou") for _ in range(2)]
                    for dvc in range(2):
                        nc.vector.tensor_copy(ou[dvc], pso[dvc])
                    pending.append((ou, acc, h, qb))
                    if len(pending) > 2:
                        p = pending.popleft()
                        emit_norm(p)
                        if p[2] == 1:
                            emit_oproj_quarter(p[3])
            while pending:
                p = pending.popleft()
                emit_norm(p)
                if p[2] == 1:
                    emit_oproj_quarter(p[3])

    nc.finalize()
    return nc


def _get_nc(causal: bool):
    key = bool(causal)
    if key not in _BUILD_CACHE:
        _BUILD_CACHE[key] = _build(causal)
    return _BUILD_CACHE[key]


def _rope_tables(position_ids_b):
    # cosT/sinT: [HD, S] fp32, transposed layout for the [d, s] dataflow
    pos = np.asarray(position_ids_b, dtype=np.float64)
    inv = 1.0 / (ROPE_BASE ** (np.arange(0, HD, 2, dtype=np.float64) / HD))
    f = pos[:, None] * inv[None, :]            # [S, HD/2]
    emb = np.concatenate([f, f], axis=1)       # [S, HD]
    cosT = np.ascontiguousarray(np.cos(emb).T.astype(np.float32))
    sinT = np.ascontiguousarray(np.sin(emb).T.astype(np.float32))
    return cosT, sinT


def _is_causal(attention_mask):
    m = np.asarray(attention_mask)
    if m.shape != (B, 1, S, S):
        return False
    tri = np.tril(np.ones((S, S), dtype=bool))
    canon = np.where(tri, np.float32(0.0), np.float32(-1e9))
    return all(np.array_equal(m[b, 0], canon) for b in range(B))


_ONES_NP = np.ones((P, P), dtype=np.float32)


def _stair():
    # multiplicative staircase: stair01[p, j] = 1 if (j - 512) >= p else 0
    j = np.arange(2 * SBLK)[None, :] - SBLK
    p = np.arange(P)[:, None]
    return np.where(j >= p, np.float32(1.0), np.float32(0.0)).astype(np.float32)


def kernel(hidden_state, attention_mask, position_ids, Wq, Wk, Wv, Wo,
           _trace=False, _tmpdir=None):
    global LAST_EXEC_TIME_NS
    hidden_state = np.asarray(hidden_state, dtype=np.float32)
    Wq = np.asarray(Wq, dtype=np.float32)
    Wk = np.asarray(Wk, dtype=np.float32)
    Wv = np.asarray(Wv, dtype=np.float32)
    Wo = np.asarray(Wo, dtype=np.float32)

    causal = _is_causal(attention_mask)
    nc = _get_nc(causal)

    stair = _stair() if causal else None
    in_maps = []
    per_batch = {}
    for b in range(B):
        hTb = np.ascontiguousarray(hidden_state[b].T)          # [D, S]
        cosT, sinT = _rope_tables(position_ids[b])
        mb = None
        if not causal:
            mb = np.ascontiguousarray(
                np.exp(np.asarray(attention_mask, dtype=np.float64)[b, 0].T)
                .astype(np.float32))
        per_batch[b] = (hTb, cosT, sinT, mb)

    for core in range(8):
        b = core // 4
        hp = core % 4
        hTb, cosT, sinT, mb = per_batch[b]
        im = {
            "hT": hTb,
            "ones": _ONES_NP,
            "wq": np.ascontiguousarray(Wq[:, hp * DQ:(hp + 1) * DQ]),
            "wk": Wk,
            "wv": Wv,
            "wo": np.ascontiguousarray(Wo[hp * DQ:(hp + 1) * DQ, :]),
            "cosT": cosT,
            "sinT": sinT,
        }
        if causal:
            im["stair"] = stair
        else:
            im["maskT16"] = mb
        in_maps.append(im)

    res = run_bass_kernel_spmd(nc, in_maps, core_ids=list(range(8)),
                               trace=_trace, tmpdir=_tmpdir)
    LAST_EXEC_TIME_NS = res.exec_time_ns

    out = np.empty((B, S, D), dtype=np.float32)
    for b in range(B):
        acc = res.results[4 * b]["out_partial"].astype(np.float32).copy()
        for hp in range(1, 4):
            acc += res.results[4 * b + hp]["out_partial"]
        out[b] = acc
    return out


# revision 18
# speedup vs baseline: 1.2057x; 1.1100x over previous
"""GemmaAttention (B=2, S=2048, D=2048, H=8, KV=1, HD=256) on 8 trn2 NeuronCores.

Sharding: DP=2 over batch x TP=4 over head-pairs. Core c handles batch c//4 and
heads {2*(c%4), 2*(c%4)+1}. Each core computes its partial o_proj output
(row-parallel Wo); the host sums the 4 partials per batch (the all-reduce is
folded into the host-side unshard).

Dataflow on each core (everything float32r on the PE at full rate):
  QT[dq,s]  = Wq_sl.T @ hT   (hT = hidden[b].T, host-transposed)
  KT[dk,s]  = Wk.T   @ hT
  V[s,dv]   = (hT chunks as lhsT) @ Wv
  RoPE applied to QT/KT in the psum->SBUF drain (DVE), with 1/sqrt(HD) folded
  into the exp's scale argument.
  scoresT[k,q] = KT_chunk.T @ QT  (per head)
  expT = ACT Exp(scoresT * 1/16) (+ causal staircase / external mask)
  outT[dv,q] += V_chunk.T @ expT ; denominators via DVE accumulation of expT
  plus a ones-vector matmul partition-reduce; normalize outT by 1/sum.
  out_partial[s,:] = outTn_chunk.T @ Wo_sl   -> DMA to DRAM.
"""

import numpy as np

import concourse.bass as bass
import concourse.tile as tile
import concourse.mybir as mybir
from concourse import bacc
from concourse.bass_utils import run_bass_kernel_spmd
from concourse._compat import with_exitstack  # noqa: F401

P = 128
B, S, D = 2, 2048, 2048
H, KV, HD = 8, 1, 256
ROPE_BASE = 10000.0
NEG_BIG = -1.0e30

HEADS_PER_CORE = 2
DQ = HEADS_PER_CORE * HD          # 512 q-dims per core
DCH = D // P                      # 16 contraction chunks
SBLK = 512                        # s-tile for projection rhs / q-tile
NSBLK = S // SBLK                 # 4
NKC = S // P                      # 16 key chunks
NQCH = DQ // P                    # 4 QT partition chunks
NKCH = HD // P                    # 2 KT partition chunks

F32 = mybir.dt.float32
F32R = mybir.dt.float32r
EXP = mybir.ActivationFunctionType.Exp

# exec time of the last traced run (set by run_spmd when tracing)
LAST_EXEC_TIME_NS = None

_BUILD_CACHE = {}


def _build(causal: bool):
    nc = bacc.Bacc()

    hT = nc.declare_dram_parameter("hT", [D, S], F32R, isOutput=False)
    wq = nc.declare_dram_parameter("wq", [D, DQ], F32R, isOutput=False)
    wk = nc.declare_dram_parameter("wk", [D, HD], F32R, isOutput=False)
    wv = nc.declare_dram_parameter("wv", [D, HD], F32R, isOutput=False)
    wo = nc.declare_dram_parameter("wo", [DQ, D], F32R, isOutput=False)
    cosT = nc.declare_dram_parameter("cosT", [HD, S], F32, isOutput=False)
    sinT = nc.declare_dram_parameter("sinT", [HD, S], F32, isOutput=False)
    ones = nc.declare_dram_parameter("ones", [P, P], F32R, isOutput=False)
    if causal:
        stair = nc.declare_dram_parameter("stair", [P, 2 * SBLK], F32, isOutput=False)
    else:
        maskT = nc.declare_dram_parameter("emaskT", [S, S], F32, isOutput=False)
    outp = nc.declare_dram_parameter("out_partial", [S, D], F32, isOutput=True)

    from contextlib import ExitStack
    with tile.TileContext(nc) as tc, ExitStack() as ctx:
        # persistent across phases: QT/KT/V + small consts
        pq = ctx.enter_context(tc.tile_pool(name="pq", bufs=1))
        QT = pq.tile([P, NQCH, S], F32R, name="QT")
        KT = pq.tile([P, NKCH, S], F32R, name="KT")
        VN = pq.tile([P, NKC, HD], F32R, name="VN")
        ONES = pq.tile([P, P], F32R, name="ONES")
        nc.sync.dma_start(out=ONES, in_=ones[:, :])
        ONEC = ONES[:, 0:1]
        ONER = ONES[0:1, :]

        # ---- phase A+B: projections + RoPE (scoped pools) -------------
        with tc.tile_pool(name="pw", bufs=1) as pw, \
             tc.tile_pool(name="pht", bufs=16) as pht, tc.tile_pool(name="pcs", bufs=2) as pcs, \
             tc.tile_pool(name="ptmp", bufs=2) as ptmp, \
             tc.tile_pool(name="ps_p", bufs=8, space="PSUM") as ps_p:
            WQ = pw.tile([P, DCH, DQ], F32R, name="WQ")
            WK = pw.tile([P, DCH, HD], F32R, name="WK")
            WV = pw.tile([P, DCH, HD], F32R, name="WV")

            for sb in range(NSBLK):
                ssl = slice(sb * SBLK, (sb + 1) * SBLK)
                # main pass: QT / KT / V(si 0,1) -- 8 psum banks exactly,
                # V reuses the same ht tiles as QK
                psq = [ps_p.tile([P, SBLK], F32, name="pp") for _ in range(NQCH)]
                psk = [ps_p.tile([P, SBLK], F32, name="pp") for _ in range(NKCH)]
                psv01 = [ps_p.tile([P, SBLK], F32, name="pp") for _ in range(2)]
                COSb = pcs.tile([P, NKCH, SBLK], F32, name="cosb")
                SINb = pcs.tile([P, NKCH, SBLK], F32, name="sinb")
                hts = []
                for c in range(DCH):
                    if sb == 0:
                        # weight chunks stream just ahead of their first use
                        nc.sync.dma_start(out=WQ[:, c, :], in_=wq[c * P:(c + 1) * P, :])
                        nc.sync.dma_start(out=WK[:, c, :], in_=wk[c * P:(c + 1) * P, :])
                        nc.sync.dma_start(out=WV[:, c, :], in_=wv[c * P:(c + 1) * P, :])
                    if 2 <= c < 2 + NKCH:
                        nc.sync.dma_start(out=COSb[:, c - 2, :],
                                          in_=cosT[(c - 2) * P:(c - 1) * P, ssl])
                        nc.sync.dma_start(out=SINb[:, c - 2, :],
                                          in_=sinT[(c - 2) * P:(c - 1) * P, ssl])
                    ht = pht.tile([P, SBLK], F32R, name="ht")
                    hts.append(ht)
                    nc.sync.dma_start(out=ht, in_=hT[c * P:(c + 1) * P, ssl])
                    for i in range(NQCH):
                        nc.tensor.matmul(psq[i], lhsT=WQ[:, c, i * P:(i + 1) * P],
                                         rhs=ht, start=(c == 0), stop=(c == DCH - 1))
                    for j in range(NKCH):
                        nc.tensor.matmul(psk[j], lhsT=WK[:, c, j * P:(j + 1) * P],
                                         rhs=ht, start=(c == 0), stop=(c == DCH - 1))
                    for si in range(2):
                        nc.tensor.matmul(psv01[si][:, :HD],
                                         lhsT=ht[:, si * P:(si + 1) * P],
                                         rhs=WV[:, c, :], start=(c == 0),
                                         stop=(c == DCH - 1))
                for si in range(2):
                    nc.vector.tensor_copy(VN[:, sb * (SBLK // P) + si, :],
                                          psv01[si][:, :HD])
                # RoPE drains (fused psum->SBUF)
                def rope_pair(p0, p1, out0, out1):
                    c0 = COSb[:, 0, :]; c1 = COSb[:, 1, :]
                    s0 = SINb[:, 0, :]; s1 = SINb[:, 1, :]
                    t1 = ptmp.tile([P, SBLK], F32, name="t")
                    t2 = ptmp.tile([P, SBLK], F32, name="t")
                    nc.vector.tensor_mul(t1, p0, c0)
                    nc.vector.tensor_mul(t2, p1, s0)
                    nc.vector.tensor_sub(out0, t1, t2)
                    t3 = ptmp.tile([P, SBLK], F32, name="t")
                    t4 = ptmp.tile([P, SBLK], F32, name="t")
                    nc.vector.tensor_mul(t3, p1, c1)
                    nc.vector.tensor_mul(t4, p0, s1)
                    nc.vector.tensor_add(out1, t3, t4)
                for h in range(HEADS_PER_CORE):
                    rope_pair(psq[2 * h], psq[2 * h + 1],
                              QT[:, 2 * h, ssl], QT[:, 2 * h + 1, ssl])
                rope_pair(psk[0], psk[1], KT[:, 0, ssl], KT[:, 1, ssl])

                # mini pass: V(si 2,3) -- reuses the resident ht tiles
                psv23 = [ps_p.tile([P, SBLK], F32, name="pp") for _ in range(2)]
                for c in range(DCH):
                    for si in range(2):
                        nc.tensor.matmul(psv23[si][:, :HD],
                                         lhsT=hts[c][:, (2 + si) * P:(3 + si) * P],
                                         rhs=WV[:, c, :], start=(c == 0),
                                         stop=(c == DCH - 1))
                for si in range(2):
                    nc.vector.tensor_copy(VN[:, sb * (SBLK // P) + 2 + si, :],
                                          psv23[si][:, :HD])

        # ---- late persistent: o_proj weights + normalized outT --------
        patt = ctx.enter_context(tc.tile_pool(name="patt", bufs=1))
        WO = patt.tile([P, NQCH, D], F32R, name="WO")
        for c in range(NQCH):
            nc.sync.dma_start(out=WO[:, c, :], in_=wo[c * P:(c + 1) * P, :])
        OUTN = patt.tile([P, NQCH, S], F32R, name="OUTN")

        # ---- phase C+D: attention + interleaved o_proj ----------------
        with tc.tile_pool(name="pexp", bufs=6) as pexp, \
             tc.tile_pool(name="pacc", bufs=4) as pacc, \
             tc.tile_pool(name="pou", bufs=8) as pou, \
             tc.tile_pool(name="pmisc", bufs=2) as pmisc, \
             tc.tile_pool(name="pmask", bufs=4) as pmask, \
             tc.tile_pool(name="pfin", bufs=3) as pfin, \
             tc.tile_pool(name="ps_s", bufs=3, space="PSUM") as ps_s, \
             tc.tile_pool(name="ps_o", bufs=2, space="PSUM") as ps_o, \
             tc.tile_pool(name="ps_r", bufs=1, space="PSUM") as ps_r, \
             tc.tile_pool(name="ps_f", bufs=2, space="PSUM") as ps_f:
            if causal:
                STAIR = pq.tile([P, 2 * SBLK], F32, name="STAIR")
                nc.sync.dma_start(out=STAIR, in_=stair[:, :])

            def emit_norm(pend):
                ou, acc, h, qb = pend
                qsl = slice(qb * SBLK, (qb + 1) * SBLK)
                pssum = ps_r.tile([P, SBLK], F32, name="pr")
                nc.tensor.matmul(pssum[0:1, :], lhsT=ONEC, rhs=acc)
                rsb = pmisc.tile([1, SBLK], F32R, name="rsb")
                with nc.allow_low_precision("f32r output is f32-width"):
                    nc.vector.reciprocal(rsb, pssum[0:1, :])
                psb = ps_r.tile([P, SBLK], F32, name="pr")
                nc.tensor.matmul(psb, lhsT=ONER, rhs=rsb)
                rbc = pmisc.tile([P, SBLK], F32R, name="rbc")
                nc.vector.tensor_copy(rbc, psb)
                for dvc in range(2):
                    nc.vector.tensor_mul(OUTN[:, 2 * h + dvc, qsl], ou[dvc], rbc)

            def emit_oproj_quarter(qb):
                # output rows s in [qb*512, (qb+1)*512) need OUTN[:, :, rows]
                for st in range(4 * qb, 4 * qb + 4):
                    stsl = slice(st * P, (st + 1) * P)
                    for nb in range(NSBLK):
                        psf = ps_f.tile([P, SBLK], F32, name="psf")
                        for dvc in range(NQCH):
                            nc.tensor.matmul(psf, lhsT=OUTN[:, dvc, stsl],
                                             rhs=WO[:, dvc, nb * SBLK:(nb + 1) * SBLK],
                                             start=(dvc == 0), stop=(dvc == NQCH - 1))
                        fsb = pfin.tile([P, SBLK], F32, name="fsb")
                        nc.scalar.copy(fsb, psf)
                        nc.sync.dma_start(out=outp[stsl, nb * SBLK:(nb + 1) * SBLK],
                                          in_=fsb)

            from collections import deque
            pending = deque()
            for h in range(HEADS_PER_CORE):
                for qb in range(NSBLK):
                    qsl = slice(qb * SBLK, (qb + 1) * SBLK)
                    klim = 4 * (qb + 1) if causal else NKC
                    pso = [ps_o.tile([P, SBLK], F32, name="pso") for _ in range(2)]
                    acc = pacc.tile([P, SBLK], F32R, name="acc")
                    for kc0 in range(0, klim, 2):
                        kcs = [kc0, kc0 + 1]
                        exs = []
                        for kc in kcs:
                            pss = ps_s.tile([P, SBLK], F32, name="pss")
                            for c in range(NKCH):
                                nc.tensor.matmul(pss,
                                                 lhsT=KT[:, c, kc * P:(kc + 1) * P],
                                                 rhs=QT[:, 2 * h + c, qsl],
                                                 start=(c == 0), stop=(c == NKCH - 1))
                            ex = pexp.tile([P, SBLK], F32R, name="ex")
                            nc.scalar.activation(ex, pss, EXP, scale=1.0 / 16.0)
                            if causal and kc >= 4 * qb:
                                delta = 128 * kc - 512 * qb
                                nc.vector.tensor_mul(ex, ex,
                                                     STAIR[:, 512 - delta:1024 - delta])
                            if not causal:
                                mt = pmask.tile([P, SBLK], F32, name="mt")
                                nc.sync.dma_start(
                                    out=mt, in_=maskT[kc * P:(kc + 1) * P, qsl])
                                nc.vector.tensor_mul(ex, ex, mt)
                            exs.append(ex)
                        for kc, ex in zip(kcs, exs):
                            if kc == 0:
                                nc.vector.tensor_copy(acc, ex)
                            else:
                                nc.vector.tensor_add(acc, acc, ex)
                        for kc, ex in zip(kcs, exs):
                            for dvc in range(2):
                                nc.tensor.matmul(pso[dvc],
                                                 lhsT=VN[:, kc, dvc * P:(dvc + 1) * P],
                                                 rhs=ex, start=(kc == 0),
                                                 stop=(kc == klim - 1))
                    # drain attn@V unnormalized so the psum slots free early
                    ou = [pou.tile([P, SBLK], F32R, name="ou") for _ in range(2)]
                    for dvc in range(2):
                        nc.vector.tensor_copy(ou[dvc], pso[dvc])
                    pending.append((ou, acc, h, qb))
                    if len(pending) > 2:
                        p = pending.popleft()
                        emit_norm(p)
                        if p[2] == 1:
                            emit_oproj_quarter(p[3])
            while pending:
                p = pending.popleft()
                emit_norm(p)
                if p[2] == 1:
                    emit_oproj_quarter(p[3])

    nc.finalize()
    return nc


def _get_nc(causal: bool):
    key = bool(causal)
    if key not in _BUILD_CACHE:
        _BUILD_CACHE[key] = _build(causal)
    return _BUILD_CACHE[key]


def _rope_tables(position_ids_b):
    # cosT/sinT: [HD, S] fp32, transposed layout for the [d, s] dataflow
    pos = np.asarray(position_ids_b, dtype=np.float64)
    inv = 1.0 / (ROPE_BASE ** (np.arange(0, HD, 2, dtype=np.float64) / HD))
    f = pos[:, None] * inv[None, :]            # [S, HD/2]
    emb = np.concatenate([f, f], axis=1)       # [S, HD]
    cosT = np.ascontiguousarray(np.cos(emb).T.astype(np.float32))
    sinT = np.ascontiguousarray(np.sin(emb).T.astype(np.float32))
    return cosT, sinT


def _is_causal(attention_mask):
    m = np.asarray(attention_mask)
    if m.shape != (B, 1, S, S):
        return False
    tri = np.tril(np.ones((S, S), dtype=bool))
    canon = np.where(tri, np.float32(0.0), np.float32(-1e9))
    return all(np.array_equal(m[b, 0], canon) for b in range(B))


_ONES_NP = np.ones((P, P), dtype=np.float32)


def _stair():
    # multiplicative staircase: stair01[p, j] = 1 if (j - 512) >= p else 0
    j = np.arange(2 * SBLK)[None, :] - SBLK
    p = np.arange(P)[:, None]
    return np.where(j >= p, np.float32(1.0), np.float32(0.0)).astype(np.float32)


def kernel(hidden_state, attention_mask, position_ids, Wq, Wk, Wv, Wo,
           _trace=False, _tmpdir=None):
    global LAST_EXEC_TIME_NS
    hidden_state = np.asarray(hidden_state, dtype=np.float32)
    Wq = np.asarray(Wq, dtype=np.float32)
    Wk = np.asarray(Wk, dtype=np.float32)
    Wv = np.asarray(Wv, dtype=np.float32)
    Wo = np.asarray(Wo, dtype=np.float32)

    causal = _is_causal(attention_mask)
    nc = _get_nc(causal)

    stair = _stair() if causal else None
    in_maps = []
    per_batch = {}
    for b in range(B):
        hTb = np.ascontiguousarray(hidden_state[b].T)          # [D, S]
        cosT, sinT = _rope_tables(position_ids[b])
        mb = None
        if not causal:
            mb = np.ascontiguousarray(
                np.exp(np.asarray(attention_mask, dtype=np.float64)[b, 0].T)
                .astype(np.float32))
        per_batch[b] = (hTb, cosT, sinT, mb)

    for core in range(8):
        b = core // 4
        hp = core % 4
        hTb, cosT, sinT, mb = per_batch[b]
        im = {
            "hT": hTb,
            "ones": _ONES_NP,
            "wq": np.ascontiguousarray(Wq[:, hp * DQ:(hp + 1) * DQ]),
            "wk": Wk,
            "wv": Wv,
            "wo": np.ascontiguousarray(Wo[hp * DQ:(hp + 1) * DQ, :]),
            "cosT": cosT,
            "sinT": sinT,
        }
        if causal:
            im["stair"] = stair
        else:
            im["maskT16"] = mb
        in_maps.append(im)

    res = run_bass_kernel_spmd(nc, in_maps, core_ids=list(range(8)),
                               trace=_trace, tmpdir=_tmpdir)
    LAST_EXEC_TIME_NS = res.exec_time_ns

    out = np.empty((B, S, D), dtype=np.float32)
    for b in range(B):
        acc = res.results[4 * b]["out_partial"].astype(np.float32).copy()
        for hp in range(1, 4):
            acc += res.results[4 * b + hp]["out_partial"]
        out[b] = acc
    return out
